# revision 15
# baseline (speedup 1.0000x reference)
"""Trainium2 Bass kernel for LuluAttention v10.

v10 vs v9 (344 us): wk moves up the SP DMA queue and the k projection piece
runs after the v pieces, so neither stalls on the startup HBM rush.


v9 vs v8 (342 us): per-core DMA only sustains ~100GB/s with all 8 cores
hammering HBM, and v8 moved 34MB. The RoPE rotate copies (5MB SBUF->SBUF)
now run on GpSimd's software DGE instead of the HW DMA queues, and x8 for
the fp8 gate projection is derived on-chip by a casting GpSimd DMA from the
bf16 x chunk (drops the separate 4MB xT8 load).


v8 vs v7 (359 us): gate projection runs plain fp8 DoubleRow (x8 @ 64*Wg8,
sigmoid absorbs the 1/64; the ~2% argument error is squashed by sigmoid's
slope to ~1% on the output -- fits the error budget and halves the gate's
instruction count); bulk weight DMAs are interleaved between the chunk-0
projection pieces so they stop stealing HBM bandwidth from the startup
critical path (all 8 cores rush ~9MB at t=0 otherwise).


v7 vs v6 (361 us): filler pieces drip between attention pairs (not bunched at
qh-block ends) so the PE never outruns the Act-engine exp ring; o_proj(c) is
deferred two chunks (agp bufs=4) so the last, largest attention chunk gets
2x filler; wq/wg split into per-head tiles so the first projection only waits
for x0 + one 0.5MB weight slice under the 8-core HBM startup rush.


v6 vs v5 (361 us): startup critical path fixed for real -- xc(0) DMA issues
before the weight bulk, cos/sin live in per-chunk tiles so chunk 0's RoPE
only waits for a 0.5MB slice, and the non-critical weight loads issue on the
Activation engine's DMA queue to drain in parallel with the SP queue.


v5 vs v3 (354 us): startup DMAs ordered by first use; gate pieces grouped to
cut Act-table swaps; fp16 RoPE tables; V projection emits vT with N=512
matmuls (16/chunk instead of 64) and PE transposes restore the straight
layout (v4's compensated-fp8 projections regressed: tensor-engine time is
~200ns per matmul *instruction*, so tripling instruction count with short
DoubleRow streams lost; reverted).

v3 changes vs v2 (374 us):
  * scores for pair p+1 issue before the PV/denominator matmuls of pair p, so
    the Act-engine exp latency hides behind score streaming (ps_sc bufs=3;
    the reciprocal broadcast shares the ps_sc ring).
  * projections of chunk c+1 and the o_proj of chunk c-1 are interleaved into
    attention(c)'s instruction stream (one piece per qh block), filling the
    PE while ACT chews exp and filling ACT (sigmoid/copies) while PE projects.
  * causal column skip: diagonal score tiles only compute/exp columns >= o*128
    (the mask multiply still covers the full width, zeroing stale fp8 data).
  * per-chunk tiles for qro/gt (pools) and kro/v8/vr8 (per-chunk tags) so the
    interleaving can't create false whole-tile write-after-read dependencies.

Numerics identical to v2: bf16 everywhere except fp8 probs (exp shifted -2)
x (fp8 v + fp8 residual) for the PV and denominator DoubleRow matmuls.
"""

import numpy as np
import ml_dtypes
from collections import deque
from contextlib import ExitStack

import concourse.bass as bass
import concourse.bacc as bacc
import concourse.tile as tile
from concourse import mybir
from concourse.bass_utils import run_bass_kernel_spmd

BF16 = ml_dtypes.bfloat16
FP8 = ml_dtypes.float8_e4m3

HIDDEN = 2048
B = 2
S_FULL = 2048
P = 128
CH = 512
QH = 4
DQ = QH * P
KT = HIDDEN // P
SCALE = 1.0 / float(np.sqrt(128.0))
ROPE_THETA = 10000.0
VS = 16.0

DR = mybir.MatmulPerfMode.DoubleRow


def build_program(S=S_FULL):
    f32 = mybir.dt.float32
    bf16 = mybir.dt.bfloat16
    fp8 = mybir.dt.float8e4
    sig = mybir.ActivationFunctionType.Sigmoid
    expf = mybir.ActivationFunctionType.Exp

    NCH = S // CH
    ST = CH // P

    nc = bacc.Bacc("TRN2", debug=False, target_bir_lowering=False)

    xT = nc.declare_dram_parameter("xT", [HIDDEN, S], bf16, False)
    wq = nc.declare_dram_parameter("wq", [HIDDEN, DQ], bf16, False)
    wk = nc.declare_dram_parameter("wk", [HIDDEN, P], bf16, False)
    wv = nc.declare_dram_parameter("wv", [HIDDEN, P], bf16, False)
    wg = nc.declare_dram_parameter("wg", [HIDDEN, DQ], fp8, False)
    wo = nc.declare_dram_parameter("wo", [DQ, HIDDEN], bf16, False)
    bg = nc.declare_dram_parameter("bg", [DQ], f32, False)
    cosT = nc.declare_dram_parameter("cosT", [P, S], mybir.dt.float16, False)
    sinT = nc.declare_dram_parameter("sinT", [P, S], mybir.dt.float16, False)
    ident = nc.declare_dram_parameter("ident", [P, P], bf16, False)
    msk = nc.declare_dram_parameter("msk", [ST, P, CH], bf16, False)
    out = nc.declare_dram_parameter("out", [S, HIDDEN], bf16, True)

    with tile.TileContext(nc) as tc, ExitStack() as ctx:
        wpool = ctx.enter_context(tc.tile_pool(name="weights", bufs=1))
        xpool = ctx.enter_context(tc.tile_pool(name="xchunks", bufs=2))
        qkv = ctx.enter_context(tc.tile_pool(name="qkv", bufs=1))
        qrop = ctx.enter_context(tc.tile_pool(name="qrop", bufs=2))
        gtp = ctx.enter_context(tc.tile_pool(name="gtp", bufs=2))
        work = ctx.enter_context(tc.tile_pool(name="work", bufs=3))
        prp = ctx.enter_context(tc.tile_pool(name="probs", bufs=3))
        agp = ctx.enter_context(tc.tile_pool(name="agp", bufs=4))
        outp = ctx.enter_context(tc.tile_pool(name="outp", bufs=3))
        ps_mm = ctx.enter_context(tc.tile_pool(name="ps_mm", bufs=2, space="PSUM"))
        ps_sc = ctx.enter_context(tc.tile_pool(name="ps_sc", bufs=3, space="PSUM"))
        ps_at = ctx.enter_context(tc.tile_pool(name="ps_at", bufs=2, space="PSUM"))
        ps_dn = ctx.enter_context(tc.tile_pool(name="ps_dn", bufs=1, space="PSUM"))

        # ---- persistent tiles ----
        wq_t = [
            wpool.tile([P, KT, P], bf16, tag=f"wq{h}", name=f"wq{h}")
            for h in range(QH)
        ]
        wk_sb = wpool.tile([P, KT, P], bf16, tag="wk")
        wv_sb = wpool.tile([P, KT, P], bf16, tag="wv")
        wg_t = [
            wpool.tile([P, KT, P], fp8, tag=f"wg{h}", name=f"wg{h}")
            for h in range(QH)
        ]
        wo_sb = wpool.tile([P, QH, HIDDEN], bf16, tag="wo")
        bg_sb = wpool.tile([P, QH], f32, tag="bg")
        cos_t = [
            wpool.tile([P, CH], mybir.dt.float16, tag=f"cos{c}", name=f"cos{c}")
            for c in range(NCH)
        ]
        sin_t = [
            wpool.tile([P, CH], mybir.dt.float16, tag=f"sin{c}", name=f"sin{c}")
            for c in range(NCH)
        ]
        msk_sb = wpool.tile([P, ST, CH], bf16, tag="msk")
        id_sb = wpool.tile([P, P], bf16, tag="ident")
        ones_pv_t = wpool.tile([P, 2, 16], fp8, tag="ones_pv")
        nc.vector.memset(ones_pv_t, 1.0)
        ones_pv = ones_pv_t[:, :, 0:1]
        ones_bc = wpool.tile([1, P], bf16, tag="ones_bc")
        nc.vector.memset(ones_bc, 1.0 / VS)
        expbias = wpool.tile([P, 1], f32, tag="expbias")
        nc.vector.memset(expbias, -2.0)

        # per-chunk persistent K/V (separate tags avoid cross-chunk WAR deps)
        kro_t = [
            qkv.tile([P, CH], bf16, tag=f"kro{c}", name=f"kro{c}")
            for c in range(NCH)
        ]
        v8_t = [
            qkv.tile([P, ST, P], fp8, tag=f"v8{c}", name=f"v8{c}")
            for c in range(NCH)
        ]
        vr8_t = [
            qkv.tile([P, ST, P], fp8, tag=f"vr8{c}", name=f"vr8{c}")
            for c in range(NCH)
        ]

        xc_t = {}
        vts_t = {}

        def load_xc(c):
            cs = slice(c * CH, (c + 1) * CH)
            xc = xpool.tile([P, KT, CH], bf16, tag="xc")
            nc.sync.dma_start(
                out=xc, in_=xT[:, cs].rearrange("(kt p) n -> p kt n", p=P)
            )
            x8c = xpool.tile([P, KT, CH], fp8, tag="x8")
            # software-DGE cast on the idle gpsimd engine: saves a 4MB HBM load
            nc.gpsimd.dma_start(out=x8c, in_=xc)
            xc_t[c] = (xc, x8c)

        def qk_piece(c, qh, qro_c):
            def run():
                cs = slice(c * CH, (c + 1) * CH)
                ps = ps_mm.tile([P, CH], f32, tag="proj")
                for kt in range(KT):
                    lhs = wq_t[qh][:, kt, :] if qh < QH else wk_sb[:, kt, :]
                    nc.tensor.matmul(
                        ps, lhs, xc_t[c][0][:, kt, :],
                        start=(kt == 0), stop=(kt == KT - 1),
                    )
                qf = work.tile([P, CH], f32, tag="qf")
                nc.scalar.copy(out=qf, in_=ps)
                rot = work.tile([P, CH], f32, tag="rot")
                nc.gpsimd.dma_start(out=rot[0:64, :], in_=qf[64:128, :])
                nc.gpsimd.dma_start(out=rot[64:128, :], in_=qf[0:64, :])
                t1 = work.tile([P, CH], f32, tag="t1")
                nc.vector.tensor_mul(t1, qf, cos_t[c])
                t2 = work.tile([P, CH], f32, tag="t2")
                nc.gpsimd.tensor_mul(t2, rot, sin_t[c])
                dst = qro_c[:, qh, :] if qh < QH else kro_t[c][:, :]
                nc.vector.tensor_add(dst, t1, t2)
            return run

        def gate_piece(c, qh, gt_c):
            def run():
                x8c = xc_t[c][1]
                ps = ps_mm.tile([P, CH], f32, tag="proj")
                for kp in range(KT // 2):
                    sl = slice(2 * kp, 2 * kp + 2)
                    nc.tensor.matmul(
                        ps, wg_t[qh][:, sl, :], x8c[:, sl, :],
                        start=(kp == 0), stop=(kp == KT // 2 - 1), perf_mode=DR,
                    )
                nc.scalar.activation(
                    out=gt_c[:, qh, :], in_=ps, func=sig,
                    bias=bg_sb[:, qh:qh + 1], scale=1.0 / 64.0,
                )
            return run

        def vt_piece(c):
            # vT [d, sq] in one N=512 accumulation (16 matmuls vs 64 for the
            # straight layout)
            def run():
                ps = ps_mm.tile([P, CH], f32, tag="proj")
                for kt in range(KT):
                    nc.tensor.matmul(
                        ps, wv_sb[:, kt, :], xc_t[c][0][:, kt, :],
                        start=(kt == 0), stop=(kt == KT - 1),
                    )
                vts = work.tile([P, CH], bf16, tag="vts")
                nc.scalar.copy(out=vts, in_=ps)
                vts_t[c] = vts
            return run

        def v_piece(c, st):
            # PE transpose back to straight [sq, d], then fp8 + residual
            def run():
                tp = ps_mm.tile([P, P], bf16, tag="proj", name="vtp")
                nc.tensor.transpose(
                    tp, vts_t[c][:, st * P:(st + 1) * P], id_sb
                )
                nc.vector.tensor_copy(out=v8_t[c][:, st, :], in_=tp)
                nc.vector.scalar_tensor_tensor(
                    out=vr8_t[c][:, st, :],
                    in0=v8_t[c][:, st, :],
                    scalar=-1.0,
                    in1=tp,
                    op0=mybir.AluOpType.mult,
                    op1=mybir.AluOpType.add,
                )
            return run

        def proj_pieces(c, qro_c, gt_c):
            # gates last: groups the sigmoids so exp<->sigmoid Act-table
            # swaps happen ~2x per chunk instead of ~4x
            return (
                [qk_piece(c, qh, qro_c) for qh in range(QH)]
                + [vt_piece(c)]
                + [v_piece(c, st) for st in range(ST)]
                + [qk_piece(c, QH, qro_c)]
                + [gate_piece(c, qh, gt_c) for qh in range(QH)]
            )

        def oproj_pieces(c, ag):
            def piece(st, hb):
                def run():
                    r0 = c * CH + st * P
                    ops = ps_mm.tile([P, CH], f32, tag="proj")
                    for dt in range(QH):
                        nc.tensor.matmul(
                            ops,
                            ag[:, dt, st * P:(st + 1) * P],
                            wo_sb[:, dt, hb * CH:(hb + 1) * CH],
                            start=(dt == 0),
                            stop=(dt == QH - 1),
                        )
                    ob = outp.tile([P, CH], bf16, tag="ob")
                    if hb % 2 == 0:
                        nc.vector.tensor_copy(out=ob, in_=ops)
                    else:
                        nc.scalar.copy(out=ob, in_=ops)
                    nc.sync.dma_start(
                        out=out[r0:r0 + P, hb * CH:(hb + 1) * CH], in_=ob
                    )
                return run
            return [piece(st, hb) for st in range(ST) for hb in range(HIDDEN // CH)]

        def attention(c, qro_c, gt_c, ag, filler):
            npairs = (c + 1) * ST // 2
            popped = 0
            ntot = len(filler)
            steps = QH * npairs  # drip filler evenly across all (qh, pair) steps
            step = 0

            def drip():
                nonlocal popped, step
                step += 1
                while popped < ntot * step // steps:
                    filler.popleft()()
                    popped += 1

            for qh in range(QH):
                def scores(pair):
                    pr2 = prp.tile([P, 2, CH], fp8, tag="pr")
                    for i in range(2):
                        t = 2 * pair + i
                        o = t - c * ST
                        lo = o * P if o > 0 else 0
                        sc_ps = ps_sc.tile([P, CH], f32, tag="sc")
                        nc.tensor.matmul(
                            sc_ps[:, lo:],
                            kro_t[t // ST][:, (t % ST) * P:(t % ST + 1) * P],
                            qro_c[:, qh, lo:],
                            start=True,
                            stop=True,
                        )
                        nc.scalar.activation(
                            out=pr2[:, i, lo:], in_=sc_ps[:, lo:], func=expf,
                            scale=SCALE, bias=expbias[:, 0:1],
                        )
                        if o >= 0:
                            # causal boundary crosses only cols [128o, 128o+128);
                            # left of that is fully masked (zeroed on gpsimd),
                            # right is fully unmasked
                            if lo > 0:
                                nc.gpsimd.memset(pr2[:, i, 0:lo], 0.0)
                            nc.vector.tensor_mul(
                                pr2[:, i, lo:lo + P],
                                pr2[:, i, lo:lo + P],
                                msk_sb[:, o, lo:lo + P],
                            )
                    return pr2

                def pv(pair, pr2, at, dn):
                    tc0 = 2 * pair
                    cc = tc0 // ST
                    vsl = slice(tc0 % ST, tc0 % ST + 2)
                    nc.tensor.matmul(
                        at, v8_t[cc][:, vsl, :], pr2,
                        start=(pair == 0), stop=False, perf_mode=DR,
                    )
                    nc.tensor.matmul(
                        at, vr8_t[cc][:, vsl, :], pr2,
                        start=False, stop=(pair == npairs - 1), perf_mode=DR,
                    )
                    nc.tensor.matmul(
                        dn, ones_pv, pr2,
                        start=(pair == 0), stop=(pair == npairs - 1), perf_mode=DR,
                    )

                at = ps_at.tile([P, CH], f32, tag="attn")
                dn = ps_dn.tile([1, CH], f32, tag="denom")
                pr_prev = scores(0)
                for pair in range(1, npairs):
                    pr_cur = scores(pair)
                    pv(pair - 1, pr_prev, at, dn)
                    drip()
                    pr_prev = pr_cur
                pv(npairs - 1, pr_prev, at, dn)
                drip()

                rc = work.tile([1, CH], f32, tag="rc")
                nc.vector.reciprocal_approx_fast(out=rc, in_=dn)
                rcb = work.tile([1, CH], bf16, tag="rcb")
                nc.vector.tensor_copy(out=rcb, in_=rc)
                bc = ps_sc.tile([P, CH], f32, tag="sc")
                nc.tensor.matmul(bc, ones_bc, rcb, start=True, stop=True)
                t3 = work.tile([P, CH], f32, tag="t3")
                nc.vector.tensor_mul(t3, at, gt_c[:, qh, :])
                nc.vector.tensor_mul(ag[:, qh, :], t3, bc)

        # ---- main schedule ----
        # critical path first on the SP queue: x chunk 0, Wq, chunk-0 rope
        # tables; the rest drains in parallel on the ACT hwdge queue
        load_xc(0)
        nc.sync.dma_start(
            out=wq_t[0],
            in_=wq[:, 0:P].rearrange("(kt p) n -> p kt n", p=P),
        )
        nc.sync.dma_start(out=cos_t[0], in_=cosT[:, 0:CH])
        nc.sync.dma_start(out=sin_t[0], in_=sinT[:, 0:CH])
        nc.sync.dma_start(out=wk_sb, in_=wk[:, :].rearrange("(kt p) n -> p kt n", p=P))
        for h in range(1, QH):
            nc.sync.dma_start(
                out=wq_t[h],
                in_=wq[:, h * P:(h + 1) * P].rearrange("(kt p) n -> p kt n", p=P),
            )
        nc.scalar.dma_start(out=wv_sb, in_=wv[:, :].rearrange("(kt p) n -> p kt n", p=P))
        nc.scalar.dma_start(out=id_sb, in_=ident[:, :])
        nc.scalar.dma_start(out=bg_sb, in_=bg[:].rearrange("(h p) -> p h", p=P))
        nc.sync.dma_start(out=msk_sb, in_=msk[:, :, :].rearrange("o p n -> p o n"))

        def _bulk():
            for h in range(QH):
                yield lambda h=h: nc.scalar.dma_start(
                    out=wg_t[h],
                    in_=wg[:, h * P:(h + 1) * P].rearrange("(kt p) n -> p kt n", p=P),
                )
            for c in range(1, NCH):
                yield lambda c=c: nc.scalar.dma_start(
                    out=cos_t[c], in_=cosT[:, c * CH:(c + 1) * CH]
                )
                yield lambda c=c: nc.scalar.dma_start(
                    out=sin_t[c], in_=sinT[:, c * CH:(c + 1) * CH]
                )
            yield lambda: nc.scalar.dma_start(
                out=wo_sb, in_=wo[:, :].rearrange("(dt p) n -> p dt n", p=P)
            )

        bulk = list(_bulk())
        qro_c = qrop.tile([P, QH, CH], bf16, tag="qro")
        gt_c = gtp.tile([P, QH, CH], bf16, tag="gt")
        pieces0 = proj_pieces(0, qro_c, gt_c)
        for i, piece in enumerate(pieces0):
            piece()
            if i < len(bulk):
                bulk[i]()
        for b in bulk[len(pieces0):]:
            b()
        pending = deque()  # (c, ag) whose o_proj is not yet emitted
        for c in range(NCH):
            filler = deque()
            qro_n = gt_n = None
            if c + 1 < NCH:
                load_xc(c + 1)
                qro_n = qrop.tile([P, QH, CH], bf16, tag="qro")
                gt_n = gtp.tile([P, QH, CH], bf16, tag="gt")
                filler.extend(proj_pieces(c + 1, qro_n, gt_n))
            # o_proj deferred ~2 chunks: the last (largest) attention chunks
            # get the most PE filler to ride out their exp-bound stretches
            n_op = 0 if c + 1 < NCH - 1 else (1 if c + 1 == NCH - 1 else 2)
            for _ in range(min(n_op, len(pending))):
                filler.extend(oproj_pieces(*pending.popleft()))
            ag = agp.tile([P, QH, CH], bf16, tag="ag")
            attention(c, qro_c, gt_c, ag, filler)
            while filler:
                filler.popleft()()
            pending.append((c, ag))
            qro_c, gt_c = qro_n, gt_n
        while pending:
            for piece in oproj_pieces(*pending.popleft()):
                piece()

    nc.finalize()
    return nc


_PROGRAMS = {}


def _get_program(S=S_FULL):
    if S not in _PROGRAMS:
        _PROGRAMS[S] = build_program(S)
    return _PROGRAMS[S]


def _host_tables(position_ids_b, S):
    pos = np.asarray(position_ids_b, dtype=np.float32)
    inv = 1.0 / (ROPE_THETA ** (np.arange(0, P, 2, dtype=np.float32) / P))
    ang = np.concatenate([inv, inv]).astype(np.float32)[:, None] * pos[None, :]
    cosT = np.cos(ang).astype(np.float16)
    sgn = np.where(np.arange(P) < 64, -1.0, 1.0).astype(np.float32)
    sinT = (np.sin(ang) * sgn[:, None]).astype(np.float16)
    return cosT, sinT


def _causal_masks():
    o = np.arange(CH // P)[:, None, None]
    r = np.arange(P)[None, :, None]
    j = np.arange(CH)[None, None, :]
    return ((P * o + r) <= j).astype(BF16)


def make_in_maps(x, position_ids, Wq, Wk, Wv, Wo, Wg, bg, S=S_FULL):
    x = np.asarray(x, dtype=np.float32)
    msk = _causal_masks()
    maps = []
    xT_b = [np.ascontiguousarray(x[b, :S].T).astype(BF16) for b in range(B)]
    tabs = [_host_tables(np.asarray(position_ids)[b, :S], S) for b in range(B)]
    Wq = np.asarray(Wq, np.float32)
    Wk = np.asarray(Wk, np.float32)
    Wv = np.asarray(Wv, np.float32)
    Wo = np.asarray(Wo, np.float32)
    Wg = np.asarray(Wg, np.float32)
    bg = np.asarray(bg, np.float32)
    for core in range(8):
        b, g = core // 4, core % 4
        cosT, sinT = tabs[b]
        maps.append({
            "xT": xT_b[b],
            "wq": np.ascontiguousarray(Wq[:, g * DQ:(g + 1) * DQ]).astype(BF16),
            "wk": np.ascontiguousarray(Wk[:, g * P:(g + 1) * P]).astype(BF16),
            "wv": np.ascontiguousarray(VS * Wv[:, g * P:(g + 1) * P]).astype(BF16),
            "wg": np.clip(
                64.0 * Wg[:, g * DQ:(g + 1) * DQ], -240, 240
            ).astype(FP8),
            "wo": np.ascontiguousarray(Wo[g * DQ:(g + 1) * DQ, :]).astype(BF16),
            "bg": np.ascontiguousarray(bg[g * DQ:(g + 1) * DQ]),
            "cosT": cosT,
            "sinT": sinT,
            "msk": msk,
            "ident": np.eye(P, dtype=BF16),
        })
    return maps


def run(inputs, S=S_FULL, trace=False, **kw):
    nc = _get_program(S)
    maps = make_in_maps(S=S, **inputs)
    res = run_bass_kernel_spmd(nc, maps, core_ids=list(range(8)), trace=trace, **kw)
    out = np.zeros((B, S, HIDDEN), np.float32)
    for core in range(8):
        out[core // 4] += np.asarray(res.results[core]["out"], np.float32)
    return out, res


def kernel(x, position_ids, Wq, Wk, Wv, Wo, Wg, bg):
    out, _ = run(dict(x=x, position_ids=position_ids, Wq=Wq, Wk=Wk, Wv=Wv,
                      Wo=Wo, Wg=Wg, bg=bg))
    return out


# revision 17
# speedup vs baseline: 1.0476x; 1.0476x over previous
"""Trainium2 Bass kernel for LuluAttention v11.

v11 vs v8 (342 us): x8 for the fp8 gate is cast on-chip from the bf16 chunk
by GpSimd software DGE (drops the 4MB xT8 HBM load), and x chunks load as
four quarter tiles so the first projection matmul waits on 512KB, not 2MB,
during the 8-core startup HBM rush.


v8 vs v7 (359 us): gate projection runs plain fp8 DoubleRow (x8 @ 64*Wg8,
sigmoid absorbs the 1/64; the ~2% argument error is squashed by sigmoid's
slope to ~1% on the output -- fits the error budget and halves the gate's
instruction count); bulk weight DMAs are interleaved between the chunk-0
projection pieces so they stop stealing HBM bandwidth from the startup
critical path (all 8 cores rush ~9MB at t=0 otherwise).


v7 vs v6 (361 us): filler pieces drip between attention pairs (not bunched at
qh-block ends) so the PE never outruns the Act-engine exp ring; o_proj(c) is
deferred two chunks (agp bufs=4) so the last, largest attention chunk gets
2x filler; wq/wg split into per-head tiles so the first projection only waits
for x0 + one 0.5MB weight slice under the 8-core HBM startup rush.


v6 vs v5 (361 us): startup critical path fixed for real -- xc(0) DMA issues
before the weight bulk, cos/sin live in per-chunk tiles so chunk 0's RoPE
only waits for a 0.5MB slice, and the non-critical weight loads issue on the
Activation engine's DMA queue to drain in parallel with the SP queue.


v5 vs v3 (354 us): startup DMAs ordered by first use; gate pieces grouped to
cut Act-table swaps; fp16 RoPE tables; V projection emits vT with N=512
matmuls (16/chunk instead of 64) and PE transposes restore the straight
layout (v4's compensated-fp8 projections regressed: tensor-engine time is
~200ns per matmul *instruction*, so tripling instruction count with short
DoubleRow streams lost; reverted).

v3 changes vs v2 (374 us):
  * scores for pair p+1 issue before the PV/denominator matmuls of pair p, so
    the Act-engine exp latency hides behind score streaming (ps_sc bufs=3;
    the reciprocal broadcast shares the ps_sc ring).
  * projections of chunk c+1 and the o_proj of chunk c-1 are interleaved into
    attention(c)'s instruction stream (one piece per qh block), filling the
    PE while ACT chews exp and filling ACT (sigmoid/copies) while PE projects.
  * causal column skip: diagonal score tiles only compute/exp columns >= o*128
    (the mask multiply still covers the full width, zeroing stale fp8 data).
  * per-chunk tiles for qro/gt (pools) and kro/v8/vr8 (per-chunk tags) so the
    interleaving can't create false whole-tile write-after-read dependencies.

Numerics identical to v2: bf16 everywhere except fp8 probs (exp shifted -2)
x (fp8 v + fp8 residual) for the PV and denominator DoubleRow matmuls.
"""

import numpy as np
import ml_dtypes
from collections import deque
from contextlib import ExitStack

import concourse.bass as bass
import concourse.bacc as bacc
import concourse.tile as tile
from concourse import mybir
from concourse.bass_utils import run_bass_kernel_spmd

BF16 = ml_dtypes.bfloat16
FP8 = ml_dtypes.float8_e4m3

HIDDEN = 2048
B = 2
S_FULL = 2048
P = 128
CH = 512
QH = 4
DQ = QH * P
KT = HIDDEN // P
SCALE = 1.0 / float(np.sqrt(128.0))
ROPE_THETA = 10000.0
VS = 16.0

DR = mybir.MatmulPerfMode.DoubleRow


def build_program(S=S_FULL):
    f32 = mybir.dt.float32
    bf16 = mybir.dt.bfloat16
    fp8 = mybir.dt.float8e4
    sig = mybir.ActivationFunctionType.Sigmoid
    expf = mybir.ActivationFunctionType.Exp

    NCH = S // CH
    ST = CH // P

    nc = bacc.Bacc("TRN2", debug=False, target_bir_lowering=False)

    xT = nc.declare_dram_parameter("xT", [HIDDEN, S], bf16, False)
    wq = nc.declare_dram_parameter("wq", [HIDDEN, DQ], bf16, False)
    wk = nc.declare_dram_parameter("wk", [HIDDEN, P], bf16, False)
    wv = nc.declare_dram_parameter("wv", [HIDDEN, P], bf16, False)
    wg = nc.declare_dram_parameter("wg", [HIDDEN, DQ], fp8, False)
    wo = nc.declare_dram_parameter("wo", [DQ, HIDDEN], bf16, False)
    bg = nc.declare_dram_parameter("bg", [DQ], f32, False)
    cosT = nc.declare_dram_parameter("cosT", [P, S], mybir.dt.float16, False)
    sinT = nc.declare_dram_parameter("sinT", [P, S], mybir.dt.float16, False)
    ident = nc.declare_dram_parameter("ident", [P, P], bf16, False)
    msk = nc.declare_dram_parameter("msk", [ST, P, CH], bf16, False)
    out = nc.declare_dram_parameter("out", [S, HIDDEN], bf16, True)

    with tile.TileContext(nc) as tc, ExitStack() as ctx:
        wpool = ctx.enter_context(tc.tile_pool(name="weights", bufs=1))
        xpool = ctx.enter_context(tc.tile_pool(name="xchunks", bufs=2))
        qkv = ctx.enter_context(tc.tile_pool(name="qkv", bufs=1))
        qrop = ctx.enter_context(tc.tile_pool(name="qrop", bufs=2))
        gtp = ctx.enter_context(tc.tile_pool(name="gtp", bufs=2))
        work = ctx.enter_context(tc.tile_pool(name="work", bufs=3))
        prp = ctx.enter_context(tc.tile_pool(name="probs", bufs=3))
        agp = ctx.enter_context(tc.tile_pool(name="agp", bufs=4))
        outp = ctx.enter_context(tc.tile_pool(name="outp", bufs=3))
        ps_mm = ctx.enter_context(tc.tile_pool(name="ps_mm", bufs=2, space="PSUM"))
        ps_sc = ctx.enter_context(tc.tile_pool(name="ps_sc", bufs=3, space="PSUM"))
        ps_at = ctx.enter_context(tc.tile_pool(name="ps_at", bufs=2, space="PSUM"))
        ps_dn = ctx.enter_context(tc.tile_pool(name="ps_dn", bufs=1, space="PSUM"))

        # ---- persistent tiles ----
        wq_t = [
            wpool.tile([P, KT, P], bf16, tag=f"wq{h}", name=f"wq{h}")
            for h in range(QH)
        ]
        wk_sb = wpool.tile([P, KT, P], bf16, tag="wk")
        wv_sb = wpool.tile([P, KT, P], bf16, tag="wv")
        wg_t = [
            wpool.tile([P, KT, P], fp8, tag=f"wg{h}", name=f"wg{h}")
            for h in range(QH)
        ]
        wo_sb = wpool.tile([P, QH, HIDDEN], bf16, tag="wo")
        bg_sb = wpool.tile([P, QH], f32, tag="bg")
        cos_t = [
            wpool.tile([P, CH], mybir.dt.float16, tag=f"cos{c}", name=f"cos{c}")
            for c in range(NCH)
        ]
        sin_t = [
            wpool.tile([P, CH], mybir.dt.float16, tag=f"sin{c}", name=f"sin{c}")
            for c in range(NCH)
        ]
        msk_sb = wpool.tile([P, ST, CH], bf16, tag="msk")
        id_sb = wpool.tile([P, P], bf16, tag="ident")
        ones_pv_t = wpool.tile([P, 2, 16], fp8, tag="ones_pv")
        nc.vector.memset(ones_pv_t, 1.0)
        ones_pv = ones_pv_t[:, :, 0:1]
        ones_bc = wpool.tile([1, P], bf16, tag="ones_bc")
        nc.vector.memset(ones_bc, 1.0 / VS)
        expbias = wpool.tile([P, 1], f32, tag="expbias")
        nc.vector.memset(expbias, -2.0)

        # per-chunk persistent K/V (separate tags avoid cross-chunk WAR deps)
        kro_t = [
            qkv.tile([P, CH], bf16, tag=f"kro{c}", name=f"kro{c}")
            for c in range(NCH)
        ]
        v8_t = [
            qkv.tile([P, ST, P], fp8, tag=f"v8{c}", name=f"v8{c}")
            for c in range(NCH)
        ]
        vr8_t = [
            qkv.tile([P, ST, P], fp8, tag=f"vr8{c}", name=f"vr8{c}")
            for c in range(NCH)
        ]

        xc_t = {}
        vts_t = {}

        def load_xc(c):
            # four quarter tiles: the first projection only waits for 512KB
            xq = []
            x8q = []
            for q in range(4):
                k0 = q * (KT // 4)
                lo = c * CH
                xt = xpool.tile([P, KT // 4, CH], bf16, tag=f"xc{q}",
                                name=f"xc{q}_{c}")
                nc.sync.dma_start(
                    out=xt,
                    in_=xT[k0 * P:(k0 + KT // 4) * P, lo:lo + CH].rearrange(
                        "(kt p) n -> p kt n", p=P
                    ),
                )
                x8t = xpool.tile([P, KT // 4, CH], fp8, tag=f"x8{q}",
                                 name=f"x8{q}_{c}")
                nc.gpsimd.dma_start(out=x8t, in_=xt)
                xq.append(xt)
                x8q.append(x8t)
            xc_t[c] = (xq, x8q)

        def qk_piece(c, qh, qro_c):
            def run():
                cs = slice(c * CH, (c + 1) * CH)
                ps = ps_mm.tile([P, CH], f32, tag="proj")
                for kt in range(KT):
                    lhs = wq_t[qh][:, kt, :] if qh < QH else wk_sb[:, kt, :]
                    nc.tensor.matmul(
                        ps, lhs, xc_t[c][0][kt // 4][:, kt % 4, :],
                        start=(kt == 0), stop=(kt == KT - 1),
                    )
                qf = work.tile([P, CH], f32, tag="qf")
                nc.scalar.copy(out=qf, in_=ps)
                rot = work.tile([P, CH], f32, tag="rot")
                nc.sync.dma_start(out=rot[0:64, :], in_=qf[64:128, :])
                nc.sync.dma_start(out=rot[64:128, :], in_=qf[0:64, :])
                t1 = work.tile([P, CH], f32, tag="t1")
                nc.vector.tensor_mul(t1, qf, cos_t[c])
                t2 = work.tile([P, CH], f32, tag="t2")
                nc.gpsimd.tensor_mul(t2, rot, sin_t[c])
                dst = qro_c[:, qh, :] if qh < QH else kro_t[c][:, :]
                nc.vector.tensor_add(dst, t1, t2)
            return run

        def gate_piece(c, qh, gt_c):
            def run():
                x8q = xc_t[c][1]
                ps = ps_mm.tile([P, CH], f32, tag="proj")
                for kp in range(KT // 2):
                    q, r = (2 * kp) // 4, (2 * kp) % 4
                    nc.tensor.matmul(
                        ps, wg_t[qh][:, 2 * kp:2 * kp + 2, :],
                        x8q[q][:, r:r + 2, :],
                        start=(kp == 0), stop=(kp == KT // 2 - 1), perf_mode=DR,
                    )
                nc.scalar.activation(
                    out=gt_c[:, qh, :], in_=ps, func=sig,
                    bias=bg_sb[:, qh:qh + 1], scale=1.0 / 64.0,
                )
            return run

        def vt_piece(c):
            # vT [d, sq] in one N=512 accumulation (16 matmuls vs 64 for the
            # straight layout)
            def run():
                ps = ps_mm.tile([P, CH], f32, tag="proj")
                for kt in range(KT):
                    nc.tensor.matmul(
                        ps, wv_sb[:, kt, :], xc_t[c][0][kt // 4][:, kt % 4, :],
                        start=(kt == 0), stop=(kt == KT - 1),
                    )
                vts = work.tile([P, CH], bf16, tag="vts")
                nc.scalar.copy(out=vts, in_=ps)
                vts_t[c] = vts
            return run

        def v_piece(c, st):
            # PE transpose back to straight [sq, d], then fp8 + residual
            def run():
                tp = ps_mm.tile([P, P], bf16, tag="proj", name="vtp")
                nc.tensor.transpose(
                    tp, vts_t[c][:, st * P:(st + 1) * P], id_sb
                )
                nc.vector.tensor_copy(out=v8_t[c][:, st, :], in_=tp)
                nc.vector.scalar_tensor_tensor(
                    out=vr8_t[c][:, st, :],
                    in0=v8_t[c][:, st, :],
                    scalar=-1.0,
                    in1=tp,
                    op0=mybir.AluOpType.mult,
                    op1=mybir.AluOpType.add,
                )
            return run

        def proj_pieces(c, qro_c, gt_c):
            # gates last: groups the sigmoids so exp<->sigmoid Act-table
            # swaps happen ~2x per chunk instead of ~4x
            return (
                [qk_piece(c, qh, qro_c) for qh in range(QH + 1)]
                + [vt_piece(c)]
                + [v_piece(c, st) for st in range(ST)]
                + [gate_piece(c, qh, gt_c) for qh in range(QH)]
            )

        def oproj_pieces(c, ag):
            def piece(st, hb):
                def run():
                    r0 = c * CH + st * P
                    ops = ps_mm.tile([P, CH], f32, tag="proj")
                    for dt in range(QH):
                        nc.tensor.matmul(
                            ops,
                            ag[:, dt, st * P:(st + 1) * P],
                            wo_sb[:, dt, hb * CH:(hb + 1) * CH],
                            start=(dt == 0),
                            stop=(dt == QH - 1),
                        )
                    ob = outp.tile([P, CH], bf16, tag="ob")
                    if hb % 2 == 0:
                        nc.vector.tensor_copy(out=ob, in_=ops)
                    else:
                        nc.scalar.copy(out=ob, in_=ops)
                    nc.sync.dma_start(
                        out=out[r0:r0 + P, hb * CH:(hb + 1) * CH], in_=ob
                    )
                return run
            return [piece(st, hb) for st in range(ST) for hb in range(HIDDEN // CH)]

        def attention(c, qro_c, gt_c, ag, filler):
            npairs = (c + 1) * ST // 2
            popped = 0
            ntot = len(filler)
            steps = QH * npairs  # drip filler evenly across all (qh, pair) steps
            step = 0

            def drip():
                nonlocal popped, step
                step += 1
                while popped < ntot * step // steps:
                    filler.popleft()()
                    popped += 1

            for qh in range(QH):
                def scores(pair):
                    pr2 = prp.tile([P, 2, CH], fp8, tag="pr")
                    for i in range(2):
                        t = 2 * pair + i
                        o = t - c * ST
                        lo = o * P if o > 0 else 0
                        sc_ps = ps_sc.tile([P, CH], f32, tag="sc")
                        nc.tensor.matmul(
                            sc_ps[:, lo:],
                            kro_t[t // ST][:, (t % ST) * P:(t % ST + 1) * P],
                            qro_c[:, qh, lo:],
                            start=True,
                            stop=True,
                        )
                        nc.scalar.activation(
                            out=pr2[:, i, lo:], in_=sc_ps[:, lo:], func=expf,
                            scale=SCALE, bias=expbias[:, 0:1],
                        )
                        if o >= 0:
                            # causal boundary crosses only cols [128o, 128o+128);
                            # left of that is fully masked (zeroed on gpsimd),
                            # right is fully unmasked
                            if lo > 0:
                                nc.gpsimd.memset(pr2[:, i, 0:lo], 0.0)
                            nc.vector.tensor_mul(
                                pr2[:, i, lo:lo + P],
                                pr2[:, i, lo:lo + P],
                                msk_sb[:, o, lo:lo + P],
                            )
                    return pr2

                def pv(pair, pr2, at, dn):
                    tc0 = 2 * pair
                    cc = tc0 // ST
                    vsl = slice(tc0 % ST, tc0 % ST + 2)
                    nc.tensor.matmul(
                        at, v8_t[cc][:, vsl, :], pr2,
                        start=(pair == 0), stop=False, perf_mode=DR,
                    )
                    nc.tensor.matmul(
                        at, vr8_t[cc][:, vsl, :], pr2,
                        start=False, stop=(pair == npairs - 1), perf_mode=DR,
                    )
                    nc.tensor.matmul(
                        dn, ones_pv, pr2,
                        start=(pair == 0), stop=(pair == npairs - 1), perf_mode=DR,
                    )

                at = ps_at.tile([P, CH], f32, tag="attn")
                dn = ps_dn.tile([1, CH], f32, tag="denom")
                pr_prev = scores(0)
                for pair in range(1, npairs):
                    pr_cur = scores(pair)
                    pv(pair - 1, pr_prev, at, dn)
                    drip()
                    pr_prev = pr_cur
                pv(npairs - 1, pr_prev, at, dn)
                drip()

                rc = work.tile([1, CH], f32, tag="rc")
                nc.vector.reciprocal_approx_fast(out=rc, in_=dn)
                rcb = work.tile([1, CH], bf16, tag="rcb")
                nc.vector.tensor_copy(out=rcb, in_=rc)
                bc = ps_sc.tile([P, CH], f32, tag="sc")
                nc.tensor.matmul(bc, ones_bc, rcb, start=True, stop=True)
                t3 = work.tile([P, CH], f32, tag="t3")
                nc.vector.tensor_mul(t3, at, gt_c[:, qh, :])
                nc.vector.tensor_mul(ag[:, qh, :], t3, bc)

        # ---- main schedule ----
        # critical path first on the SP queue: x chunk 0, Wq, chunk-0 rope
        # tables; the rest drains in parallel on the ACT hwdge queue
        load_xc(0)
        nc.sync.dma_start(
            out=wq_t[0],
            in_=wq[:, 0:P].rearrange("(kt p) n -> p kt n", p=P),
        )
        nc.sync.dma_start(out=cos_t[0], in_=cosT[:, 0:CH])
        nc.sync.dma_start(out=sin_t[0], in_=sinT[:, 0:CH])
        for h in range(1, QH):
            nc.sync.dma_start(
                out=wq_t[h],
                in_=wq[:, h * P:(h + 1) * P].rearrange("(kt p) n -> p kt n", p=P),
            )
        nc.sync.dma_start(out=wk_sb, in_=wk[:, :].rearrange("(kt p) n -> p kt n", p=P))
        nc.scalar.dma_start(out=wv_sb, in_=wv[:, :].rearrange("(kt p) n -> p kt n", p=P))
        nc.scalar.dma_start(out=id_sb, in_=ident[:, :])
        nc.scalar.dma_start(out=bg_sb, in_=bg[:].rearrange("(h p) -> p h", p=P))
        nc.sync.dma_start(out=msk_sb, in_=msk[:, :, :].rearrange("o p n -> p o n"))

        def _bulk():
            for h in range(QH):
                yield lambda h=h: nc.scalar.dma_start(
                    out=wg_t[h],
                    in_=wg[:, h * P:(h + 1) * P].rearrange("(kt p) n -> p kt n", p=P),
                )
            for c in range(1, NCH):
                yield lambda c=c: nc.scalar.dma_start(
                    out=cos_t[c], in_=cosT[:, c * CH:(c + 1) * CH]
                )
                yield lambda c=c: nc.scalar.dma_start(
                    out=sin_t[c], in_=sinT[:, c * CH:(c + 1) * CH]
                )
            yield lambda: nc.scalar.dma_start(
                out=wo_sb, in_=wo[:, :].rearrange("(dt p) n -> p dt n", p=P)
            )

        bulk = list(_bulk())
        qro_c = qrop.tile([P, QH, CH], bf16, tag="qro")
        gt_c = gtp.tile([P, QH, CH], bf16, tag="gt")
        pieces0 = proj_pieces(0, qro_c, gt_c)
        for i, piece in enumerate(pieces0):
            piece()
            if i < len(bulk):
                bulk[i]()
        for b in bulk[len(pieces0):]:
            b()
        pending = deque()  # (c, ag) whose o_proj is not yet emitted
        for c in range(NCH):
            filler = deque()
            qro_n = gt_n = None
            if c + 1 < NCH:
                load_xc(c + 1)
                qro_n = qrop.tile([P, QH, CH], bf16, tag="qro")
                gt_n = gtp.tile([P, QH, CH], bf16, tag="gt")
                filler.extend(proj_pieces(c + 1, qro_n, gt_n))
            # o_proj deferred ~2 chunks: the last (largest) attention chunks
            # get the most PE filler to ride out their exp-bound stretches
            n_op = 0 if c + 1 < NCH - 1 else (1 if c + 1 == NCH - 1 else 2)
            for _ in range(min(n_op, len(pending))):
                filler.extend(oproj_pieces(*pending.popleft()))
            ag = agp.tile([P, QH, CH], bf16, tag="ag")
            attention(c, qro_c, gt_c, ag, filler)
            while filler:
                filler.popleft()()
            pending.append((c, ag))
            qro_c, gt_c = qro_n, gt_n
        while pending:
            for piece in oproj_pieces(*pending.popleft()):
                piece()

    nc.finalize()
    return nc


_PROGRAMS = {}


def _get_program(S=S_FULL):
    if S not in _PROGRAMS:
        _PROGRAMS[S] = build_program(S)
    return _PROGRAMS[S]


def _host_tables(position_ids_b, S):
    pos = np.asarray(position_ids_b, dtype=np.float32)
    inv = 1.0 / (ROPE_THETA ** (np.arange(0, P, 2, dtype=np.float32) / P))
    ang = np.concatenate([inv, inv]).astype(np.float32)[:, None] * pos[None, :]
    cosT = np.cos(ang).astype(np.float16)
    sgn = np.where(np.arange(P) < 64, -1.0, 1.0).astype(np.float32)
    sinT = (np.sin(ang) * sgn[:, None]).astype(np.float16)
    return cosT, sinT


def _causal_masks():
    o = np.arange(CH // P)[:, None, None]
    r = np.arange(P)[None, :, None]
    j = np.arange(CH)[None, None, :]
    return ((P * o + r) <= j).astype(BF16)


def make_in_maps(x, position_ids, Wq, Wk, Wv, Wo, Wg, bg, S=S_FULL):
    x = np.asarray(x, dtype=np.float32)
    msk = _causal_masks()
    maps = []
    xT_b = [np.ascontiguousarray(x[b, :S].T).astype(BF16) for b in range(B)]
    tabs = [_host_tables(np.asarray(position_ids)[b, :S], S) for b in range(B)]
    Wq = np.asarray(Wq, np.float32)
    Wk = np.asarray(Wk, np.float32)
    Wv = np.asarray(Wv, np.float32)
    Wo = np.asarray(Wo, np.float32)
    Wg = np.asarray(Wg, np.float32)
    bg = np.asarray(bg, np.float32)
    for core in range(8):
        b, g = core // 4, core % 4
        cosT, sinT = tabs[b]
        maps.append({
            "xT": xT_b[b],
            "wq": np.ascontiguousarray(Wq[:, g * DQ:(g + 1) * DQ]).astype(BF16),
            "wk": np.ascontiguousarray(Wk[:, g * P:(g + 1) * P]).astype(BF16),
            "wv": np.ascontiguousarray(VS * Wv[:, g * P:(g + 1) * P]).astype(BF16),
            "wg": np.clip(
                64.0 * Wg[:, g * DQ:(g + 1) * DQ], -240, 240
            ).astype(FP8),
            "wo": np.ascontiguousarray(Wo[g * DQ:(g + 1) * DQ, :]).astype(BF16),
            "bg": np.ascontiguousarray(bg[g * DQ:(g + 1) * DQ]),
            "cosT": cosT,
            "sinT": sinT,
            "msk": msk,
            "ident": np.eye(P, dtype=BF16),
        })
    return maps


def run(inputs, S=S_FULL, trace=False, **kw):
    nc = _get_program(S)
    maps = make_in_maps(S=S, **inputs)
    res = run_bass_kernel_spmd(nc, maps, core_ids=list(range(8)), trace=trace, **kw)
    out = np.zeros((B, S, HIDDEN), np.float32)
    for core in range(8):
        out[core // 4] += np.asarray(res.results[core]["out"], np.float32)
    return out, res


def kernel(x, position_ids, Wq, Wk, Wv, Wo, Wg, bg):
    out, _ = run(dict(x=x, position_ids=position_ids, Wq=Wq, Wk=Wk, Wv=Wv,
                      Wo=Wo, Wg=Wg, bg=bg))
    return out
